# revision 10
# baseline (speedup 1.0000x reference)
"""Self-contained Trainium2 (Bass/Tile) kernel for the nn_Encoder problem.

kernel(**inputs) takes the FULL unsharded inputs (as produced by
setup_inputs()) and returns the FULL [4, 2048, 1024] fp32 output.

8-way data-parallel over tokens (2 NeuronCores per batch row, 1024
query-tokens each; K/V computed redundantly per pair => no collectives).

v4: v3 + activation-engine offload and software-pipelined emission.
 - every matmul stationary operand serves 2 consecutive moving chunks
   (LDWEIGHTS amortization, ~3x measured per-MM issue rate)
 - persistent pools; phase-disjoint tensors share ring slots
 - K never materialized: S^T = xn^T (Wk'^T Q); k-bias cancels in softmax
 - single-pass LN1 per 512-token chunk; x loaded in k-pair tiles
 - Act engine holds only exp/sqrt (+ y stores and W1-piece DMAs on its
   DGE queue); Q/KQ/V/AO/transpose fixups on DVE; relu split DVE/Act
   with the 1/WS fold moved to the W2 output (1/WS^2)
 - next rep's LN1 + loads are emitted between attention and FFN of the
   current rep so its stats matmuls fill the LN2 pipeline gap
 - x, xo, h, y bf16; fp8 e4m3 DoubleRow matmuls, weights prescaled 16
"""
import os
import numpy as np

import concourse.bass as bass
import concourse.bacc as bacc
import concourse.mybir as mybir
import concourse.tile as tile

F32 = mybir.dt.float32
BF16 = mybir.dt.bfloat16
F8 = mybir.dt.float8e4
AF = mybir.ActivationFunctionType
ALU = mybir.AluOpType
DR = mybir.MatmulPerfMode.DoubleRow

E = 1024
FF = 4096
B, S = 4, 2048
T = 1024      # own tokens per core
R = 2048      # row tokens (for K/V)
P = 128
NE = E // P   # 8
NT = T // P   # 8
NR = R // P   # 16
EPS = 1e-5
WS = 16.0     # weight prescale (power of 2)


def build(nc):
    # ---- DRAM I/O ----
    xrT = nc.dram_tensor("xrT", [E, R], BF16, kind="ExternalInput")   # feature-major
    xo = nc.dram_tensor("xo", [T, E], BF16, kind="ExternalInput")     # token-major own
    wq8 = nc.dram_tensor("wq8", [E // 2, 2 * E], F8, kind="ExternalInput")
    wkq8 = nc.dram_tensor("wkq8", [E // 2, 2 * E], F8, kind="ExternalInput")
    wv8 = nc.dram_tensor("wv8", [E // 2, 2 * E], F8, kind="ExternalInput")
    wo8 = nc.dram_tensor("wo8", [E // 2, 2 * E], F8, kind="ExternalInput")
    w1s = nc.dram_tensor("w1s", [(FF // P) * 4 * P, 256], F8, kind="ExternalInput")
    w28 = nc.dram_tensor("w28", [FF // 2, 2 * E], F8, kind="ExternalInput")
    bqw = nc.dram_tensor("bqw", [E], F32, kind="ExternalInput")       # bq * WS
    b1w = nc.dram_tensor("b1w", [FF], F32, kind="ExternalInput")      # b1 * WS
    bvh = nc.dram_tensor("bvh", [E], BF16, kind="ExternalInput")
    b2h = nc.dram_tensor("b2h", [E], BF16, kind="ExternalInput")
    boh = nc.dram_tensor("boh", [E], BF16, kind="ExternalInput")
    g3h = nc.dram_tensor("g3h", [E], BF16, kind="ExternalInput")
    b3h = nc.dram_tensor("b3h", [E], BF16, kind="ExternalInput")
    ident_in = nc.dram_tensor("ident_in", [P, P], BF16, kind="ExternalInput")
    onesb_in = nc.dram_tensor("onesb_in", [P, 1], BF16, kind="ExternalInput")
    ones8_in = nc.dram_tensor("ones8_in", [P, 32], F8, kind="ExternalInput")
    y = nc.dram_tensor("y", [T, E], BF16, kind="ExternalOutput")

    def bcast_ap(vec_t, n):
        a = vec_t.ap()
        return bass.AP(tensor=a.tensor, offset=a.offset, ap=[[0, P], [1, n]])

    with tile.TileContext(nc) as tc:
        consts_cm = tc.tile_pool(name="consts", bufs=1)
        consts = consts_cm.__enter__()

        ident_sb = consts.tile([P, P], BF16, tag="ident")
        nc.sync.dma_start(out=ident_sb, in_=ident_in.ap())
        onesb_sb = consts.tile([P, 1], BF16, tag="onesb")
        nc.sync.dma_start(out=onesb_sb, in_=onesb_in.ap())
        ones8_sb = consts.tile([P, 2, 16], F8, tag="ones8")
        nc.sync.dma_start(out=ones8_sb, in_=ones8_in.ap())
        bv_b = consts.tile([P, E], BF16, tag="bv_b")
        nc.sync.dma_start(out=bv_b, in_=bcast_ap(bvh, E))
        b2_b = consts.tile([P, E], BF16, tag="b2_b")
        nc.sync.dma_start(out=b2_b, in_=bcast_ap(b2h, E))
        eps_row = consts.tile([1, 1], F32, tag="eps_row")
        nc.vector.memset(eps_row, EPS)
        eps_col = consts.tile([P, 1], F32, tag="eps_col")
        nc.vector.memset(eps_col, EPS)
        bq_sb = consts.tile([P, NE], F32, tag="bq")
        nc.sync.dma_start(out=bq_sb, in_=bqw.ap().rearrange("(t p) -> p t", p=P))
        b1_sb = consts.tile([P, FF // P], F32, tag="b1")
        nc.sync.dma_start(out=b1_sb, in_=b1w.ap().rearrange("(t p) -> p t", p=P))
        bo_b = consts.tile([P, E], BF16, tag="bo_b")
        nc.sync.dma_start(out=bo_b, in_=bcast_ap(boh, E))
        g3_b = consts.tile([P, E], BF16, tag="g3_b")
        nc.sync.dma_start(out=g3_b, in_=bcast_ap(g3h, E))
        b3_b = consts.tile([P, E], BF16, tag="b3_b")
        nc.sync.dma_start(out=b3_b, in_=bcast_ap(b3h, E))

        big_cm = tc.tile_pool(name="big", bufs=1)
        big = big_cm.__enter__()
        stream_cm = tc.tile_pool(name="stream", bufs=1)
        stream = stream_cm.__enter__()
        dram_cm = tc.tile_pool(name="dram", bufs=1, space="DRAM")
        dram = dram_cm.__enter__()
        stat_d = dram.tile([1, T], F32, tag="stat_d")
        ps_mm_cm = tc.tile_pool(name="ps_mm", bufs=3, space="PSUM")
        ps_mm = ps_mm_cm.__enter__()
        ps_st_cm = tc.tile_pool(name="ps_st", bufs=1, space="PSUM")
        ps_st = ps_st_cm.__enter__()
        ps_tp_cm = tc.tile_pool(name="ps_tp", bufs=1, space="PSUM")
        ps_tp = ps_tp_cm.__enter__()

        def load_w2():
            w2_j = []
            for j in range(16):
                t = big.tile([P, 2, E], F8, tag=f"w2_{j}", bufs=1, name=f"w2{j}")
                nc.sync.dma_start(out=t, in_=w28.ap()[j * P:(j + 1) * P, :])
                w2_j.append(t)
            return w2_j

        def stage_a():
            """Weight-ring loads, LN1 (single pass, chunked), xo/xob."""
            st = {}
            wq_j, wkq_j, wv_j, wo_j = [], [], [], []
            for lst, dt_, nm in ((wq_j, wq8, "wq"), (wkq_j, wkq8, "wkq"),
                                 (wv_j, wv8, "wv"), (wo_j, wo8, "wo")):
                for j in range(4):
                    t = big.tile([P, 2, E], F8, tag="wring", bufs=8,
                                 name=f"{nm}{j}")
                    nc.sync.dma_start(out=t, in_=dt_.ap()[j * P:(j + 1) * P, :])
                    lst.append(t)
            st.update(wq=wq_j, wkq=wkq_j, wv=wv_j, wo=wo_j)

            xn8 = [big.tile([P, 2, R], F8, tag=f"xn8_{j}", bufs=1, name=f"xn8{j}")
                   for j in range(4)]
            st["xn8"] = xn8

            for c in range(4):
                cs = slice(c * 512, (c + 1) * 512)
                xk = []
                for kk in range(4):
                    xpair = stream.tile([P, 2, 512], BF16, tag="xa", bufs=4,
                                        name=f"xa{kk}_{c}")
                    nc.sync.dma_start(
                        out=xpair,
                        in_=xrT.ap()[kk * 2 * P:(kk + 1) * 2 * P, cs].rearrange(
                            "(a p) c -> p a c", a=2, p=P))
                    xk.append(xpair[:, 0, :])
                    xk.append(xpair[:, 1, :])
                ps_s = ps_st.tile([1, 512], F32, tag="ps_s", bufs=1)
                ps_q = ps_st.tile([1, 512], F32, tag="ps_q", bufs=1)
                for k in range(NE):
                    sq = stream.tile([P, 512], BF16, tag="scr", bufs=3, name="sq")
                    nc.vector.tensor_mul(sq, xk[k], xk[k])
                    nc.tensor.matmul(ps_s[:], onesb_sb[:], xk[k],
                                     start=(k == 0), stop=(k == NE - 1))
                    nc.tensor.matmul(ps_q[:], onesb_sb[:], sq[:],
                                     start=(k == 0), stop=(k == NE - 1))
                mean = stream.tile([1, 512], F32, tag="row", bufs=3, name="mean")
                nc.vector.tensor_scalar_mul(mean, ps_s[:], 1.0 / E)
                qrow = stream.tile([1, 512], F32, tag="row", bufs=3, name="qrow")
                nc.vector.tensor_scalar_mul(qrow, ps_q[:], 1.0 / E)
                msq = stream.tile([1, 512], F32, tag="row", bufs=3, name="msq")
                nc.vector.tensor_mul(msq, mean[:], mean[:])
                mrow_h = stream.tile([1, 512], BF16, tag="mrh", bufs=1, name="mrh")
                nc.vector.tensor_copy(out=mrow_h, in_=mean[:])
                var = stream.tile([1, 512], F32, tag="row", bufs=3, name="var")
                nc.vector.tensor_tensor(out=var, in0=qrow[:], in1=msq[:],
                                        op=ALU.subtract)
                sd = stream.tile([1, 512], F32, tag="row", bufs=3, name="sd")
                nc.scalar.activation(out=sd, in_=var[:], func=AF.Sqrt,
                                     bias=eps_row[:], scale=1.0)
                rstd = stream.tile([1, 512], F32, tag="row", bufs=3, name="rstd")
                nc.vector.reciprocal(rstd, sd[:])
                rrow_h = stream.tile([1, 512], BF16, tag="rrh", bufs=1, name="rrh")
                nc.vector.tensor_copy(out=rrow_h, in_=rstd[:])
                m_b = stream.tile([P, 512], BF16, tag="m_b", bufs=2, name="m_b")
                nc.gpsimd.partition_broadcast(m_b, mrow_h[:])
                r_b = stream.tile([P, 512], BF16, tag="r_b", bufs=2, name="r_b")
                nc.gpsimd.partition_broadcast(r_b, rrow_h[:])
                for k in range(NE):
                    xm = stream.tile([P, 512], BF16, tag="scr", bufs=3, name="xm")
                    nc.vector.tensor_tensor(out=xm, in0=xk[k], in1=m_b[:],
                                            op=ALU.subtract)
                    if k % 2 == 0:
                        nc.gpsimd.tensor_mul(xn8[k // 2][:, k % 2, cs], xm[:],
                                             r_b[:])
                    else:
                        nc.vector.tensor_mul(xn8[k // 2][:, k % 2, cs], xm[:],
                                             r_b[:])

            xob = []
            for tm in range(NT):
                xo_t = stream.tile([P, E], BF16, tag="xo", bufs=3, name="xo_t")
                nc.sync.dma_start(out=xo_t, in_=xo.ap()[tm * P:(tm + 1) * P, :])
                xb = stream.tile([P, E], BF16, tag="xob", bufs=4, name="xb")
                nc.gpsimd.tensor_add(xb, xo_t[:], bo_b[:])
                xob.append(xb)
            st["xob"] = xob
            return st

        def stage_b1(st):
            """Q, KQ, V, scores+exp, denominators, PV, Wo+h, LN2+transpose."""
            xn8 = st["xn8"]
            qp = [big.tile([P, 2, T], F8, tag="qh", bufs=8, name=f"qp{j}")
                  for j in range(4)]
            for m in range(NE):
                psq = [ps_mm.tile([P, 512], F32, tag="mm", name=f"psq{qc}")
                       for qc in range(2)]
                for j in range(4):
                    w = st["wq"][j][:, :, m * P:(m + 1) * P]
                    for qc in range(2):
                        qs = slice(qc * 512, (qc + 1) * 512)
                        nc.tensor.matmul(psq[qc][:], w, xn8[j][:, :, qs],
                                         perf_mode=DR,
                                         start=(j == 0), stop=(j == 3))
                for qc in range(2):
                    qs = slice(qc * 512, (qc + 1) * 512)
                    # qp = (psq + bq*WS) / WS
                    nc.vector.tensor_scalar(out=qp[m // 2][:, m % 2, qs],
                                            in0=psq[qc][:],
                                            scalar1=bq_sb[:, m:m + 1],
                                            scalar2=1.0 / WS,
                                            op0=ALU.add, op1=ALU.mult)
            kq = [big.tile([P, 2, T], F8, tag="qh", bufs=8, name=f"kq{j}")
                  for j in range(4)]
            for m in range(NE):
                psk = [ps_mm.tile([P, 512], F32, tag="mm", name=f"psk{qc}")
                       for qc in range(2)]
                for j in range(4):
                    w = st["wkq"][j][:, :, m * P:(m + 1) * P]
                    for qc in range(2):
                        qs = slice(qc * 512, (qc + 1) * 512)
                        nc.tensor.matmul(psk[qc][:], w, qp[j][:, :, qs],
                                         perf_mode=DR,
                                         start=(j == 0), stop=(j == 3))
                for qc in range(2):
                    qs = slice(qc * 512, (qc + 1) * 512)
                    nc.vector.tensor_scalar_mul(kq[m // 2][:, m % 2, qs],
                                                psk[qc][:], 1.0 / WS)
            vp = [big.tile([P, 2, E], F8, tag="vh", bufs=8, name=f"vp{j}")
                  for j in range(8)]
            for rm in range(NR):
                psv = [ps_mm.tile([P, 512], F32, tag="mm", name=f"psv{c}")
                       for c in range(2)]
                for j in range(4):
                    w = xn8[j][:, :, rm * P:(rm + 1) * P]
                    for c in range(2):
                        cs = slice(c * 512, (c + 1) * 512)
                        nc.tensor.matmul(psv[c][:], w, st["wv"][j][:, :, cs],
                                         perf_mode=DR,
                                         start=(j == 0), stop=(j == 3))
                for c in range(2):
                    cs = slice(c * 512, (c + 1) * 512)
                    nc.vector.scalar_tensor_tensor(
                        out=vp[rm // 2][:, rm % 2, cs], in0=psv[c][:],
                        scalar=1.0 / WS, in1=bv_b[:, cs],
                        op0=ALU.mult, op1=ALU.add)

            expp = [big.tile([P, 2, T], F8, tag="eg", bufs=16, name=f"ex{j}")
                    for j in range(8)]
            for kt in range(NR):
                pss = [ps_mm.tile([P, 512], F32, tag="mm", name=f"pss{qc}")
                       for qc in range(2)]
                for j in range(4):
                    w = xn8[j][:, :, kt * P:(kt + 1) * P]
                    for qc in range(2):
                        qs = slice(qc * 512, (qc + 1) * 512)
                        nc.tensor.matmul(pss[qc][:], w, kq[j][:, :, qs],
                                         perf_mode=DR,
                                         start=(j == 0), stop=(j == 3))
                for qc in range(2):
                    qs = slice(qc * 512, (qc + 1) * 512)
                    nc.scalar.activation(out=expp[kt // 2][:, kt % 2, qs],
                                         in_=pss[qc][:], func=AF.Exp,
                                         scale=1.0 / 32.0)
            den_row = stream.tile([1, T], F32, tag="den_row", bufs=1)
            ps_d = [ps_st.tile([1, 512], F32, tag=f"ps_d{qc}", bufs=1,
                               name=f"ps_d{qc}") for qc in range(2)]
            for jj in range(8):
                for qc in range(2):
                    qs = slice(qc * 512, (qc + 1) * 512)
                    nc.tensor.matmul(ps_d[qc][:], ones8_sb[:, :, 0:1],
                                     expp[jj][:, :, qs], perf_mode=DR,
                                     start=(jj == 0), stop=(jj == 7))
            for qc in range(2):
                qs = slice(qc * 512, (qc + 1) * 512)
                rcp = stream.tile([1, 512], F32, tag="rcp", bufs=1, name="rcp")
                nc.vector.reciprocal(rcp, ps_d[qc][:])
                # fold: /WS for Wo weights, *8 for AO/8 fp8 copy
                nc.vector.tensor_scalar_mul(den_row[:, qs], rcp[:], 8.0 / WS)
            recip_col = stream.tile([P, NT], F32, tag="recip_col", bufs=2)
            nc.sync.dma_start(out=stat_d[:], in_=den_row[:])
            nc.sync.dma_start(out=recip_col,
                              in_=stat_d[:].rearrange("a (t p) -> (a p) t", p=P))

            aop = [big.tile([P, 2, T], F8, tag="qh", bufs=8, name=f"ao{j}")
                   for j in range(4)]
            h_t = [big.tile([P, E], BF16, tag="vh", bufs=8, name=f"h{t}")
                   for t in range(NT)]
            for m in range(NE):
                psa = [ps_mm.tile([P, 512], F32, tag="mm", name=f"psa{qc}")
                       for qc in range(2)]
                for j in range(8):
                    w = vp[j][:, :, m * P:(m + 1) * P]
                    for qc in range(2):
                        qs = slice(qc * 512, (qc + 1) * 512)
                        nc.tensor.matmul(psa[qc][:], w, expp[j][:, :, qs],
                                         perf_mode=DR,
                                         start=(j == 0), stop=(j == 7))
                for qc in range(2):
                    qs = slice(qc * 512, (qc + 1) * 512)
                    # AO/8 into fp8 (range safety); folded back via recip
                    nc.vector.tensor_scalar_mul(aop[m // 2][:, m % 2, qs],
                                                psa[qc][:], 0.125)
            for tm in range(NT):
                pso = [ps_mm.tile([P, 512], F32, tag="mm", name=f"pso{c}")
                       for c in range(2)]
                for j in range(4):
                    w = aop[j][:, :, tm * P:(tm + 1) * P]
                    for c in range(2):
                        cs = slice(c * 512, (c + 1) * 512)
                        nc.tensor.matmul(pso[c][:], w, st["wo"][j][:, :, cs],
                                         perf_mode=DR,
                                         start=(j == 0), stop=(j == 3))
                for c in range(2):
                    cs = slice(c * 512, (c + 1) * 512)
                    nc.vector.scalar_tensor_tensor(
                        out=h_t[tm][:, cs], in0=pso[c][:],
                        scalar=recip_col[:, tm:tm + 1], in1=st["xob"][tm][:, cs],
                        op0=ALU.mult, op1=ALU.add)
            st["h_t"] = h_t

            hp = [big.tile([P, 2, T], F8, tag="qh", bufs=8, name=f"hp{j}")
                  for j in range(4)]
            for tm in range(NT):
                st2 = stream.tile([P, 2, 6], F32, tag="st2", bufs=3, name="st2")
                hg = h_t[tm][:].rearrange("p (g d) -> p g d", g=2)
                for g in range(2):
                    nc.vector.bn_stats(out=st2[:, g, :], in_=hg[:, g, :])
                mv2 = stream.tile([P, 2], F32, tag="mv2", bufs=3, name="mv2")
                nc.vector.bn_aggr(out=mv2, in_=st2[:])
                sd2 = stream.tile([P, 1], F32, tag="sd2", bufs=3, name="sd2")
                nc.scalar.activation(out=sd2, in_=mv2[:, 1:2], func=AF.Sqrt,
                                     bias=eps_col[:], scale=1.0)
                rinv2 = stream.tile([P, 1], F32, tag="rinv2", bufs=3, name="rinv2")
                nc.vector.reciprocal(rinv2, sd2[:])
                hn = stream.tile([P, E], BF16, tag="hn", bufs=2, name="hn")
                nc.vector.tensor_scalar(out=hn, in0=h_t[tm][:],
                                        scalar1=mv2[:, 0:1], scalar2=rinv2[:],
                                        op0=ALU.subtract, op1=ALU.mult)
                for g in range(2):
                    tp = ps_tp.tile([P, 512], BF16, tag="tp", name="tp")
                    for i in range(4):
                        et = 4 * g + i
                        nc.tensor.transpose(tp[:, i * P:(i + 1) * P],
                                            hn[:, et * P:(et + 1) * P],
                                            ident_sb[:])
                    for jj in range(2):
                        j = 2 * g + jj
                        nc.vector.tensor_copy(
                            out=hp[j][:, :, tm * P:(tm + 1) * P],
                            in_=tp[:, jj * 256:(jj + 1) * 256].rearrange(
                                "p (a b) -> p a b", a=2))
            st["hp"] = hp

        def stage_b2(st, load_next_w2):
            """FFN (W1 streamed via Act DGE), LN3, y; then next rep's w2."""
            hp, h_t, w2_j = st["hp"], st["h_t"], st["w2"]
            gp = [big.tile([P, 2, T], F8, tag="eg", bufs=16, name=f"gp{j}")
                  for j in range(16)]
            for f in range(FF // P):
                w1q = stream.tile([P, 4, 2, P], F8, tag="w1s", bufs=8,
                                  name=f"w1q{f}")
                nc.scalar.dma_start(
                    out=w1q,
                    in_=w1s.ap()[f * 4 * P:(f + 1) * 4 * P, :].rearrange(
                        "(a p) c -> p a c", a=4, p=P))
                psg = [ps_mm.tile([P, 512], F32, tag="mm", name=f"psg{qc}")
                       for qc in range(2)]
                for j in range(4):
                    for qc in range(2):
                        qs = slice(qc * 512, (qc + 1) * 512)
                        nc.tensor.matmul(psg[qc][:], w1q[:, j, :, :],
                                         hp[j][:, :, qs], perf_mode=DR,
                                         start=(j == 0), stop=(j == 3))
                # gp holds WS*G (un-scaled at the W2 output instead)
                nc.scalar.activation(out=gp[f // 2][:, f % 2, 0:512],
                                     in_=psg[0][:], func=AF.Relu,
                                     bias=b1_sb[:, f:f + 1], scale=1.0)
                nc.vector.tensor_scalar(out=gp[f // 2][:, f % 2, 512:1024],
                                        in0=psg[1][:],
                                        scalar1=b1_sb[:, f:f + 1], scalar2=0.0,
                                        op0=ALU.add, op1=ALU.max)
            for tm in range(NT):
                psf = [ps_mm.tile([P, 512], F32, tag="mm", name=f"psf{c}")
                       for c in range(2)]
                hb2 = stream.tile([P, E], BF16, tag="hb2", bufs=2, name="hb2")
                nc.vector.tensor_tensor(out=hb2, in0=h_t[tm][:], in1=b2_b[:],
                                        op=ALU.add)
                for j in range(16):
                    w = gp[j][:, :, tm * P:(tm + 1) * P]
                    for c in range(2):
                        cs = slice(c * 512, (c + 1) * 512)
                        nc.tensor.matmul(psf[c][:], w, w2_j[j][:, :, cs],
                                         perf_mode=DR,
                                         start=(j == 0), stop=(j == 15))
                op = stream.tile([P, E], BF16, tag="op", bufs=2, name="op")
                for c in range(2):
                    cs = slice(c * 512, (c + 1) * 512)
                    nc.vector.scalar_tensor_tensor(out=op[:, cs], in0=psf[c][:],
                                                   scalar=1.0 / (WS * WS),
                                                   in1=hb2[:, cs],
                                                   op0=ALU.mult, op1=ALU.add)
                st3 = stream.tile([P, 2, 6], F32, tag="st3", bufs=3, name="st3")
                og = op[:].rearrange("p (g d) -> p g d", g=2)
                for g in range(2):
                    nc.vector.bn_stats(out=st3[:, g, :], in_=og[:, g, :])
                mv3 = stream.tile([P, 2], F32, tag="mv3", bufs=3, name="mv3")
                nc.vector.bn_aggr(out=mv3, in_=st3[:])
                sd3 = stream.tile([P, 1], F32, tag="sd3", bufs=3, name="sd3")
                nc.scalar.activation(out=sd3, in_=mv3[:, 1:2], func=AF.Sqrt,
                                     bias=eps_col[:], scale=1.0)
                rinv3 = stream.tile([P, 1], F32, tag="rinv3", bufs=3, name="rinv3")
                nc.vector.reciprocal(rinv3, sd3[:])
                n = stream.tile([P, E], BF16, tag="ychain", bufs=2, name="n")
                nc.vector.tensor_scalar(out=n, in0=op[:], scalar1=mv3[:, 0:1],
                                        scalar2=rinv3[:], op0=ALU.subtract,
                                        op1=ALU.mult)
                yg = stream.tile([P, E], BF16, tag="ychain", bufs=2, name="yg")
                nc.gpsimd.tensor_mul(yg, n[:], g3_b[:])
                yt = stream.tile([P, E], BF16, tag="ychain", bufs=2, name="yt")
                nc.gpsimd.tensor_add(yt, yg[:], b3_b[:])
                nc.scalar.dma_start(out=y.ap()[tm * P:(tm + 1) * P, :], in_=yt[:])
            if load_next_w2:
                return load_w2()
            return None

        reps = int(os.environ.get("ENC_REPS", "1"))
        st = stage_a()
        st["w2"] = load_w2()
        for r in range(reps):
            stage_b1(st)
            st_next = stage_a() if r + 1 < reps else None
            w2_next = stage_b2(st, load_next_w2=(st_next is not None))
            if st_next is not None:
                st_next["w2"] = w2_next
            st = st_next

        ps_tp_cm.__exit__(None, None, None)
        ps_st_cm.__exit__(None, None, None)
        ps_mm_cm.__exit__(None, None, None)
        dram_cm.__exit__(None, None, None)
        stream_cm.__exit__(None, None, None)
        big_cm.__exit__(None, None, None)
        consts_cm.__exit__(None, None, None)


# ======================= host-side prep / assembly =========================

def prep_inputs(inputs):
    import ml_dtypes
    F8NP = ml_dtypes.float8_e4m3
    BF16NP = ml_dtypes.bfloat16
    src = np.asarray(inputs["src_embs"], np.float32)   # [B, S, E]
    g1 = np.asarray(inputs["g1"], np.float32)
    b1ln = np.asarray(inputs["b1"], np.float32)
    g2 = np.asarray(inputs["g2"], np.float32)
    b2ln = np.asarray(inputs["b2"], np.float32)

    Wq, bq = np.asarray(inputs["Wq_w"], np.float32), np.asarray(inputs["Wq_b"], np.float32)
    Wk = np.asarray(inputs["Wk_w"], np.float32)
    Wv, bv = np.asarray(inputs["Wv_w"], np.float32), np.asarray(inputs["Wv_b"], np.float32)
    Wo, bo = np.asarray(inputs["Wo_w"], np.float32), np.asarray(inputs["Wo_b"], np.float32)
    W1, b1f = np.asarray(inputs["W1_w"], np.float32), np.asarray(inputs["W1_b"], np.float32)
    W2, b2f = np.asarray(inputs["W2_w"], np.float32), np.asarray(inputs["W2_b"], np.float32)

    def pairize(WT):
        # WT [K, M] fp32 -> quantized fp8 pair layout [K//2, 2*M]
        # row r = j*128+p, col = i*M+m  with k = 256j + 128i + p
        K, M = WT.shape
        W8 = (WT * WS).astype(F8NP)
        arr = W8.reshape(K // 256, 2, P, M).transpose(0, 2, 1, 3).reshape(K // 2, 2 * M)
        return np.ascontiguousarray(arr)

    wq8 = pairize((Wq * g1[None, :]).T)
    # KQ trick: contraction over Q's output features; k-bias cancels.
    wkq8 = pairize(Wk * g1[None, :])
    wv8 = pairize((Wv * g1[None, :]).T)
    wo8 = pairize(Wo.T)
    w1p = pairize((W1 * g2[None, :]).T)     # [E//2, 2*FF]
    w28 = pairize(W2.T)

    # W1 pieces: [(f*4 + j)*128 + p, i*128 + c] = w1p[j*128+p, i*4096 + f*128 + c]
    w1s = np.ascontiguousarray(
        w1p.reshape(4, P, 2, FF // P, P).transpose(3, 0, 1, 2, 4).reshape(
            (FF // P) * 4 * P, 256))

    bq_eff = (bq + Wq @ b1ln).astype(np.float32)
    bv_eff = (bv + Wv @ b1ln).astype(np.float32)
    b1_eff = (b1f + W1 @ b2ln).astype(np.float32)

    shared = dict(
        wq8=wq8, wkq8=wkq8, wv8=wv8, wo8=wo8, w1s=w1s, w28=w28,
        bqw=(bq_eff * WS).astype(np.float32),
        b1w=(b1_eff * WS).astype(np.float32),
        bvh=bv_eff.astype(BF16NP),
        b2h=b2f.astype(BF16NP),
        boh=bo.astype(BF16NP),
        g3h=np.asarray(inputs["g3"], np.float32).astype(BF16NP),
        b3h=np.asarray(inputs["b3"], np.float32).astype(BF16NP),
        ident_in=np.eye(P, dtype=BF16NP),
        onesb_in=np.ones((P, 1), BF16NP),
        ones8_in=np.ones((P, 32), F8NP),
    )
    in_maps = []
    for c in range(8):
        b, half = c // 2, c % 2
        row = src[b]
        own = row[half * T:(half + 1) * T]
        other = row[(1 - half) * T:(2 - half) * T]
        xr = np.concatenate([own, other], axis=0)
        m = dict(shared)
        m["xrT"] = np.ascontiguousarray(xr.T.astype(BF16NP))
        m["xo"] = np.ascontiguousarray(own.astype(BF16NP))
        in_maps.append(m)
    return in_maps


def assemble_output(results):
    out = np.zeros((B, S, E), np.float32)
    for c in range(8):
        b, half = c // 2, c % 2
        out[b, half * T:(half + 1) * T] = results[c]["y"]
    return out


def build_nc():
    nc = bacc.Bacc("TRN2", target_bir_lowering=False, debug=False)
    build(nc)
    nc.compile()
    return nc


_CACHE = {}


def _get_nc():
    if "nc" not in _CACHE:
        _CACHE["nc"] = build_nc()
    return _CACHE["nc"]


def kernel(**inputs):
    from concourse import bass_utils
    nc = _get_nc()
    in_maps = prep_inputs(inputs)
    res = bass_utils.run_bass_kernel_spmd(nc, in_maps, core_ids=list(range(8)))
    return assemble_output(res.results)
